# revision 28
# baseline (speedup 1.0000x reference)
"""Trainium2 Bass kernel for CONCH zero-shot top-j pooling.

Sharding: 8 slabs = (batch b, {x_s, x_l}) -> core 2b / 2b+1. Each core
streams its [20000, 512] f32 slab from HBM (cast to bf16 inline by the
SWDGE DMA), computes per-patch max dot-product against the 64 descriptor
embeddings on the tensor engine, and writes the per-patch raw max score.
The host then takes a generous top-MARGIN candidate set per batch
(validated: the true top-100 normalized scores always fall well inside
it), rescores only those candidates exactly in fp32, and finishes the
tiny pooling/softmax math.
"""

import numpy as np

B = 4
N = 20000           # patches per slab
D = 512
T = 64              # NUM_CLASSES * DESC_PER_CLASS
NUM_CLASSES = 4
DESC_PER_CLASS = 16
PT = 128            # patches per tile (SBUF partitions)
NT = (N + PT - 1) // PT     # 157 tiles
NC = D // PT        # 4 contraction chunks
G = 8               # tiles per input DMA (2 MiB fp32 reads)
MARGIN = 2048
COPY_ENGINE = "dve"
TRANSPOSE_MODE = "pe"   # "pe" = tensor-engine transpose + copy; "dma" = xbar

_CACHE = {}


def _build_nc():
    import concourse.bacc as bacc
    import concourse.mybir as mybir
    import concourse.tile as tile

    f32 = mybir.dt.float32
    bf16 = mybir.dt.bfloat16
    AF = mybir.ActivationFunctionType
    AX = mybir.AxisListType

    nc = bacc.Bacc(None)
    x = nc.dram_tensor("x", [N, D], f32, kind="ExternalInput")
    wt = nc.dram_tensor("wt", [D, T], bf16, kind="ExternalInput")
    ident = nc.dram_tensor("ident", [PT, PT], bf16, kind="ExternalInput")
    m_out = nc.dram_tensor("m_out", [PT, NT], f32, kind="ExternalOutput")

    with tile.TileContext(nc) as tc:
        with (
            tc.tile_pool(name="singles", bufs=1) as singles,
            tc.tile_pool(name="xin", bufs=5) as xin,
            tc.tile_pool(name="xtp", bufs=4) as xtp,
            tc.tile_pool(name="ps_t", bufs=4, space="PSUM") as ps_t,
            tc.tile_pool(name="ps_s", bufs=4, space="PSUM") as ps_s,
        ):
            wt_sb = singles.tile([PT, NC * T], bf16)
            nc.sync.dma_start(out=wt_sb, in_=wt.rearrange("(k q) t -> q k t", q=PT))
            id_sb = singles.tile([PT, PT], bf16)
            nc.sync.dma_start(out=id_sb, in_=ident[:, :])
            m_buf = singles.tile([PT, NT], f32)

            def pair_body(xb_a, xb_b, ti):
                # two full tiles: transpose both into one PSUM bank, one
                # split copy (ACT: tile A, DVE: tile B), 8 matmuls, 1 reduce
                h = PT
                ps_x = ps_t.tile([PT, 2 * D], bf16, tag="ps_x")
                for k, col in enumerate((xb_a, xb_b)):
                    for c in range(NC):
                        nc.tensor.transpose(
                            ps_x[:, k * D + c * h:k * D + (c + 1) * h],
                            col[:h, c * PT:(c + 1) * PT],
                            id_sb[:h, :h],
                        )
                xt = xtp.tile([PT, 2 * D], bf16, tag="xt")
                if COPY_ENGINE == "split":
                    nc.scalar.activation(xt[:, :D], ps_x[:, :D], AF.Copy)
                    nc.vector.tensor_copy(xt[:, D:], ps_x[:, D:])
                elif COPY_ENGINE == "dve":
                    nc.vector.tensor_copy(xt[:, :], ps_x[:, :])
                else:
                    nc.scalar.activation(xt[:, :], ps_x[:, :], AF.Copy)
                sc = ps_s.tile([PT, 2, T], f32, tag="sc")
                for k in range(2):
                    for c in range(NC):
                        nc.tensor.matmul(
                            sc[:h, k, :],
                            lhsT=xt[:, k * D + c * h:k * D + c * h + h],
                            rhs=wt_sb[:, c * T:(c + 1) * T],
                            start=(c == 0),
                            stop=(c == NC - 1),
                        )
                nc.vector.reduce_max(out=m_buf[:h, ti:ti + 2], in_=sc[:h, :, :],
                                     axis=AX.X)

            def tile_body(xb_col, h, ti):
                # single (tail) tile
                xt = xtp.tile([PT, 2 * D], bf16, tag="xt")
                ps_x = ps_t.tile([PT, 2 * D], bf16, tag="ps_x")
                for c in range(NC):
                    nc.tensor.transpose(
                        ps_x[:, c * h:(c + 1) * h],
                        xb_col[:h, c * PT:(c + 1) * PT],
                        id_sb[:h, :h],
                    )
                nc.scalar.activation(xt[:, :NC * h], ps_x[:, :NC * h], AF.Copy)
                sc = ps_s.tile([PT, 2, T], f32, tag="sc")
                for c in range(NC):
                    nc.tensor.matmul(
                        sc[:h, 0, :],
                        lhsT=xt[:, c * h:c * h + h],
                        rhs=wt_sb[:, c * T:(c + 1) * T],
                        start=(c == 0),
                        stop=(c == NC - 1),
                    )
                nc.vector.reduce_max(out=m_buf[:h, ti:ti + 1], in_=sc[:h, 0, :],
                                     axis=AX.X)

            # Striped layout: partition q owns patches [q*L, (q+1)*L) so every
            # DMA's per-partition run is contiguous in DRAM (big descriptors).
            L = N // PT                 # 156 stripe tiles
            tail = N - L * PT           # 32 leftover patches
            xs = x[:L * PT, :].rearrange("(q j) d -> q j d", q=PT)

            # tail first, so the kernel doesn't end on it
            if tail:
                xb = xin.tile([PT, G * D], bf16, tag="xb")
                nc.gpsimd.dma_start(out=xb[:tail, :D], in_=x[L * PT:, :])
                tile_body(xb[:, :D], tail, NT - 1)
                for p0 in range(tail, PT, 32):
                    nc.vector.memset(m_buf[p0:p0 + 32, NT - 1:NT], 0.0)

            # ramp-up/ramp-down group sizes: small first groups fill the
            # pipeline fast; small last groups shrink the compute tail
            sizes = [4, 4] + [G] * ((L - 12) // G) + [2, 2]
            assert sum(sizes) == L and all(s % 2 == 0 for s in sizes)
            t0 = 0
            for g in sizes:
                xb = xin.tile([PT, G * D], bf16, tag="xb")
                nc.gpsimd.dma_start(out=xb[:, :g * D], in_=xs[:, t0:t0 + g, :])
                for j in range(0, g, 2):
                    pair_body(xb[:, j * D:(j + 1) * D],
                              xb[:, (j + 1) * D:(j + 2) * D], t0 + j)
                t0 += g

            nc.sync.dma_start(out=m_out[:, :], in_=m_buf[:, :])
    return nc


def get_nc():
    if "nc" not in _CACHE:
        nc = _build_nc()
        nc.finalize()   # Bacc: legalize sync waits, alloc regs, freeze
        _CACHE["nc"] = nc
    return _CACHE["nc"]


def _device_scores(x_s, x_l, desc_feats, run_kwargs=None):
    """Run the 8-core SPMD kernel; returns approx raw max-dot scores [B, 2N]
    and the BassKernelResults."""
    import ml_dtypes
    from concourse.bass_utils import run_bass_kernel_spmd

    nc = get_nc()
    wt_b = np.ascontiguousarray(
        np.asarray(desc_feats, dtype=np.float32).T
    ).astype(ml_dtypes.bfloat16)
    idm = np.eye(PT, dtype=ml_dtypes.bfloat16)
    in_maps = []
    for b in range(B):
        for slab in (x_s[b], x_l[b]):
            in_maps.append({
                "x": np.ascontiguousarray(slab, dtype=np.float32),
                "wt": wt_b,
                "ident": idm,
            })
    res = run_bass_kernel_spmd(
        nc, in_maps, core_ids=list(range(2 * B)), **(run_kwargs or {})
    )
    L = N // PT
    tail = N - L * PT

    def decode(mo):
        s = np.empty(N, np.float32)
        s[:L * PT] = mo[:, :L].reshape(-1)
        if tail:
            s[L * PT:] = mo[:tail, NT - 1]
        return s

    m = np.empty((B, 2 * N), np.float32)
    for b in range(B):
        m[b, :N] = decode(res.results[2 * b]["m_out"])
        m[b, N:] = decode(res.results[2 * b + 1]["m_out"])
    return m, res


def _host_finish(x_s, x_l, desc_feats, logit_scale, topj, m):
    """Exact fp32 finish on the top-MARGIN candidates per batch."""
    desc = np.asarray(desc_feats, dtype=np.float32)
    topj = min(int(topj), 2 * N)
    margin = min(2 * N, max(MARGIN, 4 * topj))
    image_features = np.empty((B, D), np.float32)
    for b in range(B):
        cand = np.argpartition(-m[b], margin - 1)[:margin]
        cand.sort()  # ascending original index => stable tie-break like top_k
        xs_mask = cand < N
        xc = np.empty((cand.size, D), np.float32)
        xc[xs_mask] = x_s[b][cand[xs_mask]]
        xc[~xs_mask] = x_l[b][cand[~xs_mask] - N]
        nrm = np.maximum(np.linalg.norm(xc, axis=-1, keepdims=True),
                         np.float32(1e-12)).astype(np.float32)
        xn = xc / nrm
        raw = xn @ desc.T
        s = np.maximum(raw.max(-1), np.float32(0.0))
        keep = np.argsort(-s, kind="stable")[:topj]
        mean = xn[keep].mean(axis=0, dtype=np.float32)
        mnrm = max(np.linalg.norm(mean), np.float32(1e-12))
        image_features[b] = mean / mnrm
    class_text = desc.reshape(NUM_CLASSES, DESC_PER_CLASS, D).mean(
        axis=1, dtype=np.float32)
    scale = np.exp(np.asarray(logit_scale, dtype=np.float32)[0])
    logits = (image_features @ class_text.T) * scale
    z = logits - logits.max(axis=-1, keepdims=True)
    e = np.exp(z)
    Y_prob = (e / e.sum(axis=-1, keepdims=True)).astype(np.float32)
    Y_hat = np.argmax(Y_prob, axis=-1).astype(np.int32)
    return Y_prob, Y_hat


def kernel(x_s, coord_s, x_l, coord_l, desc_feats, logit_scale, topj):
    x_s = np.asarray(x_s, dtype=np.float32)
    x_l = np.asarray(x_l, dtype=np.float32)
    m, _ = _device_scores(x_s, x_l, desc_feats)
    return _host_finish(x_s, x_l, desc_feats, logit_scale, topj, m)


# revision 32
# speedup vs baseline: 1.1037x; 1.1037x over previous
"""Trainium2 Bass kernel for CONCH zero-shot top-j pooling.

Sharding: 8 slabs = (batch b, {x_s, x_l}) -> core 2b / 2b+1. Each core
streams its [20000, 512] f32 slab from HBM (cast to bf16 inline by the
SWDGE DMA), computes per-patch max dot-product against the 64 descriptor
embeddings on the tensor engine, and writes the per-patch raw max score.
The host then takes a generous top-MARGIN candidate set per batch
(validated: the true top-100 normalized scores always fall well inside
it), rescores only those candidates exactly in fp32, and finishes the
tiny pooling/softmax math.
"""

import numpy as np

B = 4
N = 20000           # patches per slab
D = 512
T = 64              # NUM_CLASSES * DESC_PER_CLASS
NUM_CLASSES = 4
DESC_PER_CLASS = 16
PT = 128            # patches per tile (SBUF partitions)
NT = (N + PT - 1) // PT     # 157 tiles
NC = D // PT        # 4 contraction chunks
G = 8               # tiles per input DMA (2 MiB fp32 reads)
MARGIN = 2048
COPY_ENGINE = "dve"
TRANSPOSE_MODE = "pe"   # "pe" = tensor-engine transpose + copy; "dma" = xbar

_CACHE = {}


def _build_nc():
    import concourse.bacc as bacc
    import concourse.mybir as mybir
    import concourse.tile as tile

    f32 = mybir.dt.float32
    bf16 = mybir.dt.bfloat16
    AF = mybir.ActivationFunctionType
    AX = mybir.AxisListType

    nc = bacc.Bacc(None)
    x = nc.dram_tensor("x", [N, D], f32, kind="ExternalInput")
    wt = nc.dram_tensor("wt", [D, T], bf16, kind="ExternalInput")
    ident = nc.dram_tensor("ident", [PT, PT], bf16, kind="ExternalInput")
    m_out = nc.dram_tensor("m_out", [PT, NT], f32, kind="ExternalOutput")

    with tile.TileContext(nc) as tc:
        with (
            tc.tile_pool(name="singles", bufs=1) as singles,
            tc.tile_pool(name="xin", bufs=5) as xin,
            tc.tile_pool(name="xtp", bufs=4) as xtp,
            tc.tile_pool(name="ps_t", bufs=4, space="PSUM") as ps_t,
            tc.tile_pool(name="ps_s", bufs=4, space="PSUM") as ps_s,
        ):
            wt_sb = singles.tile([PT, NC * T], bf16)
            nc.sync.dma_start(out=wt_sb, in_=wt.rearrange("(k q) t -> q k t", q=PT))
            id_sb = singles.tile([PT, PT], bf16)
            nc.sync.dma_start(out=id_sb, in_=ident[:, :])
            m_buf = singles.tile([PT, NT], f32)

            def pair_body(xb_a, xb_b, ti):
                # two full tiles: 8 transposes into one PSUM bank, one
                # PSUM->SBUF copy, 8 matmuls, 1 paired reduce
                h = PT
                ps_x = ps_t.tile([PT, 2 * D], bf16, tag="ps_x")
                for k, col in enumerate((xb_a, xb_b)):
                    for c in range(NC):
                        nc.tensor.transpose(
                            ps_x[:, k * D + c * h:k * D + (c + 1) * h],
                            col[:h, c * PT:(c + 1) * PT],
                            id_sb[:h, :h],
                        )
                xt = xtp.tile([PT, 2 * D], bf16, tag="xt")
                if COPY_ENGINE == "split":
                    nc.scalar.activation(xt[:, :D], ps_x[:, :D], AF.Copy)
                    nc.vector.tensor_copy(xt[:, D:], ps_x[:, D:])
                elif COPY_ENGINE == "dve":
                    nc.vector.tensor_copy(xt[:, :], ps_x[:, :])
                else:
                    nc.scalar.activation(xt[:, :], ps_x[:, :], AF.Copy)
                sc = ps_s.tile([PT, 2, T], f32, tag="sc")
                for k in range(2):
                    for c in range(NC):
                        nc.tensor.matmul(
                            sc[:h, k, :],
                            lhsT=xt[:, k * D + c * h:k * D + c * h + h],
                            rhs=wt_sb[:, c * T:(c + 1) * T],
                            start=(c == 0),
                            stop=(c == NC - 1),
                        )
                nc.vector.reduce_max(out=m_buf[:h, ti:ti + 2], in_=sc[:h, :, :],
                                     axis=AX.X)

            def tile_body(xb_col, h, ti):
                # single (tail) tile
                xt = xtp.tile([PT, 2 * D], bf16, tag="xt")
                ps_x = ps_t.tile([PT, 2 * D], bf16, tag="ps_x")
                for c in range(NC):
                    nc.tensor.transpose(
                        ps_x[:, c * h:(c + 1) * h],
                        xb_col[:h, c * PT:(c + 1) * PT],
                        id_sb[:h, :h],
                    )
                nc.scalar.activation(xt[:, :NC * h], ps_x[:, :NC * h], AF.Copy)
                sc = ps_s.tile([PT, 2, T], f32, tag="sc")
                for c in range(NC):
                    nc.tensor.matmul(
                        sc[:h, 0, :],
                        lhsT=xt[:, c * h:c * h + h],
                        rhs=wt_sb[:, c * T:(c + 1) * T],
                        start=(c == 0),
                        stop=(c == NC - 1),
                    )
                nc.vector.reduce_max(out=m_buf[:h, ti:ti + 1], in_=sc[:h, 0, :],
                                     axis=AX.X)

            # Striped layout: partition q owns patches [q*L, (q+1)*L) so every
            # DMA's per-partition run is contiguous in DRAM (big descriptors).
            L = N // PT                 # 156 stripe tiles
            tail = N - L * PT           # 32 leftover patches
            xs = x[:L * PT, :].rearrange("(q j) d -> q j d", q=PT)

            # tail first, so the kernel doesn't end on it
            if tail:
                xb = xin.tile([PT, G * D], bf16, tag="xb")
                nc.gpsimd.dma_start(out=xb[:tail, :D], in_=x[L * PT:, :])
                tile_body(xb[:, :D], tail, NT - 1)
                for p0 in range(tail, PT, 32):
                    nc.vector.memset(m_buf[p0:p0 + 32, NT - 1:NT], 0.0)

            # ramp-up/ramp-down group sizes: small first groups fill the
            # pipeline fast; small last groups shrink the compute tail
            sizes = [4, 4] + [G] * ((L - 12) // G) + [2, 2]
            assert sum(sizes) == L and all(s % 2 == 0 for s in sizes)
            t0 = 0
            mid_flush = 0
            for g in sizes:
                xb = xin.tile([PT, G * D], bf16, tag="xb")
                nc.gpsimd.dma_start(out=xb[:, :g * D], in_=xs[:, t0:t0 + g, :])
                for j in range(0, g, 2):
                    pair_body(xb[:, j * D:(j + 1) * D],
                              xb[:, (j + 1) * D:(j + 2) * D], t0 + j)
                t0 += g
                if mid_flush == 0 and t0 >= 128:
                    # flush the finished score columns while compute continues,
                    # so the final output DMA is tiny
                    nc.sync.dma_start(out=m_out[:, :t0], in_=m_buf[:, :t0])
                    mid_flush = t0

            nc.sync.dma_start(out=m_out[:, mid_flush:], in_=m_buf[:, mid_flush:])
    return nc


def get_nc():
    if "nc" not in _CACHE:
        nc = _build_nc()
        nc.finalize()   # Bacc: legalize sync waits, alloc regs, freeze
        _CACHE["nc"] = nc
    return _CACHE["nc"]


def _device_scores(x_s, x_l, desc_feats, run_kwargs=None):
    """Run the 8-core SPMD kernel; returns approx raw max-dot scores [B, 2N]
    and the BassKernelResults."""
    import ml_dtypes
    from concourse.bass_utils import run_bass_kernel_spmd

    nc = get_nc()
    wt_b = np.ascontiguousarray(
        np.asarray(desc_feats, dtype=np.float32).T
    ).astype(ml_dtypes.bfloat16)
    idm = np.eye(PT, dtype=ml_dtypes.bfloat16)
    in_maps = []
    for b in range(B):
        for slab in (x_s[b], x_l[b]):
            in_maps.append({
                "x": np.ascontiguousarray(slab, dtype=np.float32),
                "wt": wt_b,
                "ident": idm,
            })
    res = run_bass_kernel_spmd(
        nc, in_maps, core_ids=list(range(2 * B)), **(run_kwargs or {})
    )
    L = N // PT
    tail = N - L * PT

    def decode(mo):
        s = np.empty(N, np.float32)
        s[:L * PT] = mo[:, :L].reshape(-1)
        if tail:
            s[L * PT:] = mo[:tail, NT - 1]
        return s

    m = np.empty((B, 2 * N), np.float32)
    for b in range(B):
        m[b, :N] = decode(res.results[2 * b]["m_out"])
        m[b, N:] = decode(res.results[2 * b + 1]["m_out"])
    return m, res


def _host_finish(x_s, x_l, desc_feats, logit_scale, topj, m):
    """Exact fp32 finish on the top-MARGIN candidates per batch."""
    desc = np.asarray(desc_feats, dtype=np.float32)
    topj = min(int(topj), 2 * N)
    margin = min(2 * N, max(MARGIN, 4 * topj))
    image_features = np.empty((B, D), np.float32)
    for b in range(B):
        cand = np.argpartition(-m[b], margin - 1)[:margin]
        cand.sort()  # ascending original index => stable tie-break like top_k
        xs_mask = cand < N
        xc = np.empty((cand.size, D), np.float32)
        xc[xs_mask] = x_s[b][cand[xs_mask]]
        xc[~xs_mask] = x_l[b][cand[~xs_mask] - N]
        nrm = np.maximum(np.linalg.norm(xc, axis=-1, keepdims=True),
                         np.float32(1e-12)).astype(np.float32)
        xn = xc / nrm
        raw = xn @ desc.T
        s = np.maximum(raw.max(-1), np.float32(0.0))
        keep = np.argsort(-s, kind="stable")[:topj]
        mean = xn[keep].mean(axis=0, dtype=np.float32)
        mnrm = max(np.linalg.norm(mean), np.float32(1e-12))
        image_features[b] = mean / mnrm
    class_text = desc.reshape(NUM_CLASSES, DESC_PER_CLASS, D).mean(
        axis=1, dtype=np.float32)
    scale = np.exp(np.asarray(logit_scale, dtype=np.float32)[0])
    logits = (image_features @ class_text.T) * scale
    z = logits - logits.max(axis=-1, keepdims=True)
    e = np.exp(z)
    Y_prob = (e / e.sum(axis=-1, keepdims=True)).astype(np.float32)
    Y_hat = np.argmax(Y_prob, axis=-1).astype(np.int32)
    return Y_prob, Y_hat


def kernel(x_s, coord_s, x_l, coord_l, desc_feats, logit_scale, topj):
    x_s = np.asarray(x_s, dtype=np.float32)
    x_l = np.asarray(x_l, dtype=np.float32)
    m, _ = _device_scores(x_s, x_l, desc_feats)
    return _host_finish(x_s, x_l, desc_feats, logit_scale, topj, m)
